# revision 57
# baseline (speedup 1.0000x reference)
"""Multi-head attention (B=2, S=2048, DM=1024, H=16, DH=64, causal) on 8 TRN2 cores.

Sharding: tensor-parallel over heads. Core c owns heads {2c, 2c+1} = q/k/v dims
[128c, 128c+128). Each core computes its QKV projections, causal attention for
its 2 heads (both batches), and a partial output projection (row-parallel over
Wo). Host unshards by summing the 8 partials and adding bo (the TP all-reduce).

v2 layout/schedule (per core), all matmuls bf16 with fp32 PSUM accumulation:
  - xT DMA'd per 512-seq chunk so QKV proj starts ~1.5us in, not after the
    full 8MB load.
  - attention q-blocks interleaved between QKV chunks (block (b,qb) only
    needs chunks <= its key span), overlapping exp/copies with proj matmuls.
  - scores software-pipelined one k-tile ahead of AV/sums so PE doesn't wait
    on the exp activation.
  - per q-block tail inlined: one reciprocal over the (33,512) sums PSUM
    region, partition-broadcast via a single K=33 selector matmul, and the
    normalization fused into the PSUM->SBUF ctx copy.
  - causal diag-tile masking on the (otherwise idle) Pool engine.

Causality is hardcoded (the reference's attention_mask is always triu causal).
"""

import os
import sys

import numpy as np

try:
    import concourse  # noqa: F401
except ImportError:
    sys.path.insert(0, "/opt/trn_rl_repo")

import ml_dtypes

BF16 = ml_dtypes.bfloat16

B, S, DM = 2, 2048, 1024
H, DH = 16, 64
NCORES = 8
CPC = DM // NCORES  # 128 q/k/v dims per core (2 heads)
BS = B * S  # 4096
Q_W = 512  # query-block width

_CACHE = {}
LAST_EXEC_NS = None
LAST_RESULTS = None


def _build(repeat=1):
    # dev-only ablation switches for benchmarking (unset in grading)
    skip = set(os.environ.get("BENCH_SKIP", "").split(",")) - {""}
    import concourse.mybir as mybir
    from concourse import bacc
    from concourse import tile
    from concourse.masks import make_upper_triangular

    f32 = mybir.dt.float32
    f16 = mybir.dt.float16
    bf16 = mybir.dt.bfloat16
    Exp = mybir.ActivationFunctionType.Exp
    Ident = mybir.ActivationFunctionType.Identity

    nc = bacc.Bacc(
        "TRN2",
        target_bir_lowering=False,
        debug=False,
        enable_asserts=False,
        num_devices=NCORES,
    )

    xT = nc.dram_tensor("xT", (DM, BS), bf16, kind="ExternalInput").ap()
    # weights pre-packed on host to (p, t, m): contiguous 2KB partition lines
    wq = nc.dram_tensor("wq", (128, DM // 128, CPC), bf16,
                        kind="ExternalInput").ap()
    wk = nc.dram_tensor("wk", (128, DM // 128, CPC), bf16,
                        kind="ExternalInput").ap()
    wv = nc.dram_tensor("wv", (128, DM // 128, CPC), bf16,
                        kind="ExternalInput").ap()
    wo = nc.dram_tensor("wo", (CPC, DM), bf16, kind="ExternalInput").ap()
    bq = nc.dram_tensor("bq", (CPC, 1), f32, kind="ExternalInput").ap()
    out = nc.dram_tensor("out", (BS, DM), f16, kind="ExternalOutput").ap()

    KT_FEAT = DM // 128  # 8 contraction tiles
    N_CH = BS // 512  # 8 seq chunks (512 each)
    NQB = S // Q_W  # q-blocks per batch

    with tile.TileContext(nc) as tc:
      with tc.tile_pool(name="consts", bufs=1) as consts, \
           tc.tile_pool(name="sb", bufs=2) as sb, \
           tc.tile_pool(name="psp", bufs=1, space="PSUM") as psp:

        def body():
            # ---- persistent tiles ------------------------------------------
            wq_sb = consts.tile((128, KT_FEAT, CPC), bf16, name="wq_sb")
            wk_sb = consts.tile((128, KT_FEAT, CPC), bf16, name="wk_sb")
            wv_sb = consts.tile((128, KT_FEAT, CPC), bf16, name="wv_sb")
            wo_sb = consts.tile((CPC, DM), bf16, name="wo_sb")
            bq_sb = consts.tile((CPC, 1), f32, name="bq_sb")
            nc.sync.dma_start(wq_sb[:], wq)
            nc.sync.dma_start(wk_sb[:], wk)
            nc.sync.dma_start(wv_sb[:], wv)
            nc.sync.dma_start(bq_sb[:], bq)

            # ones rows for the K=1 reciprocal-broadcast matmuls (partition
            # bases must be 32-aligned, so the two rec rows live at 0 and 32)
            ones33 = consts.tile((33, 128), bf16, name="ones33")
            nc.vector.memset(ones33[:], 1.0)
            # causal mask tile: mask[k, q] = 1.0 if k <= q else 0.0
            cmask = consts.tile((128, 128), bf16, name="cmask")
            make_upper_triangular(nc, cmask[:], val=1.0, diag=True)

            QT_sb = consts.tile((128, BS), bf16, name="QT_sb")
            KT_sb = consts.tile((128, BS), bf16, name="KT_sb")
            # per-head V with a 65th ones-column: the AV matmul (M=65) then
            # emits the softmax denominator as output row 64 for free
            V0_sb = consts.tile((128, BS // 128, 65), bf16, name="V0_sb")
            V1_sb = consts.tile((128, BS // 128, 65), bf16, name="V1_sb")
            nc.vector.memset(V0_sb[:, :, 64:65], 1.0)
            nc.vector.memset(V1_sb[:, :, 64:65], 1.0)

            xT3 = xT.rearrange("(t p) q -> p t q", p=128)
            xT_sb = consts.tile((128, KT_FEAT, BS), bf16, name="xT_sb")
            for ch in range(N_CH):
                c0 = ch * 512
                nc.sync.dma_start(
                    xT_sb[:, :, c0:c0 + 512], xT3[:, :, c0:c0 + 512],
                )
            # wo isn't needed until the first tail — load after the x chunks
            nc.sync.dma_start(wo_sb[:], wo)

            def qkv_steps(ch):
                """The chunk's projection work as 6 independently-emittable
                steps (q chain, k chain, 4 V sub-chunks) so the scheduler can
                weave them between the previous attention block's tiles."""
                if "qkv" in skip:
                    def z():
                        if ch == 0:
                            nc.vector.memset(QT_sb[:], 0.01)
                            nc.vector.memset(KT_sb[:], 0.01)
                            nc.vector.memset(V0_sb[:, :, 0:64], 0.01)
                            nc.vector.memset(V1_sb[:, :, 0:64], 0.01)
                    return [z]
                c0 = ch * 512
                steps = []

                # bk is dropped: softmax over keys is invariant to the
                # per-query constant q.bk it contributes to every score.
                def qk_step(pname, w_sb, b_sb, out_T):
                    ps_p = psp.tile((128, 512), f32, name=f"ps_{pname}{ch}",
                                    tag="proj", bufs=2)
                    for t in range(KT_FEAT):
                        nc.tensor.matmul(
                            ps_p[:, :],
                            lhsT=w_sb[:, t, :],
                            rhs=xT_sb[:, t, c0:c0 + 512],
                            start=(t == 0),
                            stop=(t == KT_FEAT - 1),
                        )
                    if b_sb is not None:
                        nc.scalar.activation(
                            out_T[:, c0:c0 + 512], ps_p[:, :], Ident,
                            bias=b_sb[:],
                        )
                    else:
                        nc.scalar.activation(
                            out_T[:, c0:c0 + 512], ps_p[:, :], Ident,
                        )

                # V directly in natural (seq, vdim) layout: x chunk as the
                # stationary operand, wv moving — no PE transposes needed
                def v_step(sub):
                    s0 = c0 + sub * 128
                    ps_v = psp.tile((128, 128), f32, name=f"ps_v{ch}_{sub}",
                                    tag="proj", bufs=2)
                    for t in range(KT_FEAT):
                        nc.tensor.matmul(
                            ps_v[:, :],
                            lhsT=xT_sb[:, t, s0:s0 + 128],
                            rhs=wv_sb[:, t, :],
                            start=(t == 0),
                            stop=(t == KT_FEAT - 1),
                        )
                    # bv is dropped here: attention rows sum to 1 after
                    # normalization, so bv's contribution is the constant
                    # row bv @ Wo^T — folded into the host-side bias add.
                    nc.scalar.activation(
                        V0_sb[:, ch * 4 + sub, 0:64], ps_v[:, 0:64], Ident,
                    )
                    nc.scalar.activation(
                        V1_sb[:, ch * 4 + sub, 0:64], ps_v[:, 64:128], Ident,
                    )

                steps.append(lambda: qk_step("q", wq_sb, bq_sb, QT_sb))
                steps.append(lambda: qk_step("k", wk_sb, None, KT_sb))
                for sub in range(4):
                    steps.append(lambda sub=sub: v_step(sub))
                return steps

            def attn_block(b, qb, weave_steps, pending_tail):
                """Scores/exp/AV for one q-block, with the NEXT chunk's QKV
                steps and the PREVIOUS block's tail woven between tiles (late
                blocks are exp/Act-heavy while QKV is PE-heavy, so weaving
                keeps both engines fed). Returns the tail closure (normalize
                + output projection), which the caller hands to the next
                block so the cross-engine reciprocal chain never head-of-
                line-blocks the PE/DVE queues."""
                qb0 = qb * Q_W
                g0 = b * S + qb0
                n_t = (qb0 + Q_W) // 128  # causal: k-tiles needed

                # one (65,512) accumulator per head: rows 0-63 ctx, row 64
                # the softmax denominator (ones-column of V{h}_sb)
                ps_ctx0 = psp.tile((65, Q_W), f32, name=f"ps_ctx0_{b}_{qb}",
                                   tag="ctx0", bufs=1)
                ps_ctx1 = psp.tile((65, Q_W), f32, name=f"ps_ctx1_{b}_{qb}",
                                   tag="ctx1", bufs=1)
                ps_ctx = (ps_ctx0, ps_ctx1)

                # software pipeline: scores(t) one tile ahead of AV/sums(t-1)
                exps = [None] * n_t

                def scores_tile(t):
                    k0 = 128 * t
                    off = max(0, k0 - qb0)
                    w = Q_W - off
                    diag = k0 >= qb0
                    ps_s = psp.tile((128, 1024), f32,
                                    name=f"ps_s{b}_{qb}_{t}",
                                    tag="scores", bufs=2)
                    exp_sb = sb.tile((128, 1024), bf16,
                                     name=f"exp{b}_{qb}_{t}",
                                     tag="exp", bufs=4)
                    for h in range(2):
                        nc.tensor.matmul(
                            ps_s[:, h * 512:h * 512 + w],
                            lhsT=KT_sb[h * 64:(h + 1) * 64,
                                       b * S + k0:b * S + k0 + 128],
                            rhs=QT_sb[h * 64:(h + 1) * 64,
                                      g0 + off:g0 + Q_W],
                            start=True,
                            stop=True,
                            tile_position=(h * 64, 0),
                            skip_group_check=True,
                        )
                    # per-head exp instructions: h0's AV can start while h1's
                    # exp still runs (a single merged instruction measured
                    # 13us slower on HW)
                    if off == 0:
                        nc.scalar.activation(
                            exp_sb[:, :1024], ps_s[:, :1024], Exp,
                            scale=0.125,
                        )
                    else:
                        nc.scalar.activation(
                            exp_sb[:, :w], ps_s[:, :w], Exp, scale=0.125,
                        )
                        nc.scalar.activation(
                            exp_sb[:, 512:512 + w], ps_s[:, 512:512 + w],
                            Exp, scale=0.125,
                        )
                    if diag:  # triangular mask on the diagonal block (Pool)
                        for h in range(2):
                            nc.gpsimd.tensor_mul(
                                exp_sb[:, h * 512:h * 512 + 128],
                                exp_sb[:, h * 512:h * 512 + 128],
                                cmask[:],
                            )
                    exps[t] = exp_sb

                def av_tile(t):
                    k0 = 128 * t
                    off = max(0, k0 - qb0)
                    w = Q_W - off
                    first = t == 0
                    last = t == n_t - 1
                    exp_sb = exps[t]
                    for h, v_sb in ((0, V0_sb), (1, V1_sb)):
                        nc.tensor.matmul(
                            ps_ctx[h][0:65, off:Q_W],
                            lhsT=v_sb[:, (b * S + k0) // 128, :],
                            rhs=exp_sb[:, h * 512:h * 512 + w],
                            start=first,
                            stop=last,
                        )

                if "attn" in skip:
                    if pending_tail is not None:
                        pending_tail()
                    nc.vector.memset(ps_ctx0[0:65, :], 1.0)
                    nc.vector.memset(ps_ctx1[0:65, :], 1.0)
                    for s in weave_steps:
                        s()
                else:
                    # previous tail FIRST: its cu-mul reads the (bufs=1) ctx
                    # accumulators this block's AV is about to overwrite
                    scores_tile(0)
                    if pending_tail is not None:
                        pending_tail()
                    for t in range(1, n_t):
                        scores_tile(t)
                        av_tile(t - 1)
                        if t % 2 == 0 and weave_steps:
                            weave_steps.pop(0)()
                    av_tile(n_t - 1)
                    for s in weave_steps:  # leftovers on short blocks
                        s()

                # denominators live at row 64 of each head's accumulator
                rec = sb.tile((33, Q_W), bf16, name=f"rec{b}_{qb}",
                              tag="rec", bufs=2)
                with nc.allow_low_precision(
                    reason="bf16 softmax denominators: 2^-8 rounding on "
                           "a uniform per-(head,query) scale"
                ):
                    nc.vector.reciprocal(rec[0:1, :], ps_ctx0[64:65, :])
                    nc.vector.reciprocal(rec[32:33, :], ps_ctx1[64:65, :])

                def tail():
                    if "tail" in skip:
                        return
                    ps_bc = psp.tile((128, Q_W), f32, name=f"ps_bc{b}_{qb}",
                                     tag="scores", bufs=2)
                    nc.tensor.matmul(
                        ps_bc[0:64, :], lhsT=ones33[0:1, 0:64],
                        rhs=rec[0:1, :],
                        start=True, stop=True, tile_position=(0, 0),
                        skip_group_check=True,
                    )
                    nc.tensor.matmul(
                        ps_bc[64:128, :], lhsT=ones33[32:33, 64:128],
                        rhs=rec[32:33, :],
                        start=True, stop=True, tile_position=(32, 64),
                        skip_group_check=True,
                    )
                    rb = sb.tile((128, Q_W), f32, name=f"rb{b}_{qb}",
                                 tag="rb", bufs=2)
                    nc.scalar.activation(rb[:], ps_bc[:], Ident)
                    cu = sb.tile((128, Q_W), bf16, name=f"cu{b}_{qb}",
                                 tag="cu", bufs=2)
                    nc.vector.tensor_mul(cu[0:64, :], ps_ctx0[0:64, :],
                                         rb[0:64, :])
                    nc.vector.tensor_mul(cu[64:128, :], ps_ctx1[0:64, :],
                                         rb[64:128, :])

                    for sub in range(Q_W // 128):
                        o_sb = sb.tile((128, DM), f16,
                                       name=f"o_sb{b}_{qb}_{sub}",
                                       tag="o_sb", bufs=3)
                        for nn in range(2):
                            ps_o = psp.tile((128, 512), f32,
                                            name=f"ps_o{b}_{qb}_{sub}_{nn}",
                                            tag="proj", bufs=2)
                            nc.tensor.matmul(
                                ps_o[:, :],
                                lhsT=cu[:, sub * 128:(sub + 1) * 128],
                                rhs=wo_sb[:, nn * 512:(nn + 1) * 512],
                                start=True,
                                stop=True,
                            )
                            if "ocopy" not in skip:
                                nc.vector.tensor_copy(
                                    o_sb[:, nn * 512:(nn + 1) * 512],
                                    ps_o[:, :],
                                )
                        if "ocopy" in skip:
                            continue
                        r0 = g0 + sub * 128
                        nc.sync.dma_start(out[r0:r0 + 128, :], o_sb[:])

                return tail

            # ---- interleaved schedule: tail(i) lands after qkv(i+1)
            # (weaving qkv steps INTO the attention tile loop measured
            # ~8us slower on HW than this block-granular order) ----------
            pending_tail = None
            for ch in range(N_CH):
                for s in qkv_steps(ch):
                    s()
                if pending_tail is not None:
                    pending_tail()
                b, qb = divmod(ch, NQB)
                pending_tail = attn_block(b, qb, [], None)
            pending_tail()

        if repeat == 1:
            body()
        else:
            with tc.For_i(0, repeat, 1):
                body()

    nc.compile()
    return nc


def _pack_w(W, sl):
    """(DM, DM) torch-layout weight -> core slice, (p, t, m) packed so each
    SBUF partition line is one contiguous 2KB DMA descriptor."""
    wT = np.asarray(W, np.float32)[sl, :].T  # (DM feat, CPC out)
    return np.ascontiguousarray(
        wT.reshape(DM // 128, 128, CPC).transpose(1, 0, 2)
    ).astype(BF16)


def _prep_inputs(x, Wq, bq, Wk, bk, Wv, bv, Wo):
    """Build the 8 per-core input maps (host-side sharding). bk/bv are
    folded out: bk cancels in the softmax, bv is added host-side."""
    x = np.asarray(x, dtype=np.float32)
    xT = np.ascontiguousarray(x.reshape(BS, DM).T).astype(BF16)
    in_maps = []
    for c in range(NCORES):
        sl = slice(c * CPC, (c + 1) * CPC)
        in_maps.append({
            "xT": xT,
            "wq": _pack_w(Wq, sl),
            "wk": _pack_w(Wk, sl),
            "wv": _pack_w(Wv, sl),
            "wo": np.ascontiguousarray(np.asarray(Wo, np.float32)[:, sl].T).astype(BF16),
            "bq": np.asarray(bq, np.float32)[sl].reshape(CPC, 1).copy(),
        })
    return in_maps


def _run(in_maps, trace=False):
    global LAST_EXEC_NS, LAST_RESULTS
    from concourse import bass_utils

    if "nc" not in _CACHE:
        _CACHE["nc"] = _build()
    nc = _CACHE["nc"]
    res = bass_utils.run_bass_kernel_spmd(
        nc, in_maps, core_ids=list(range(NCORES)), trace=trace,
    )
    LAST_EXEC_NS = getattr(res, "exec_time_ns", None)
    LAST_RESULTS = res
    return res.results


def kernel(x, Wq, bq, Wk, bk, Wv, bv, Wo, bo, attention_mask=None, _trace=False):
    """Full inputs in, full output out. attention_mask is the reference's
    causal mask; causality is hardcoded in the kernel."""
    in_maps = _prep_inputs(x, Wq, bq, Wk, bk, Wv, bv, Wo)
    results = _run(in_maps, trace=_trace)
    acc = np.zeros((BS, DM), dtype=np.float32)
    for c in range(NCORES):
        acc += results[c]["out"].astype(np.float32)
    # bo plus the folded-out V bias: attn rows sum to 1, so bv contributes
    # the constant row bv @ Wo^T to every output row
    acc += (np.asarray(bo, np.float32)
            + np.asarray(Wo, np.float32) @ np.asarray(bv, np.float32))[None, :]
    return acc.reshape(B, S, DM)
